# revision 30
# baseline (speedup 1.0000x reference)
"""DeepseekV2 MLA prefill attention on 8 Trainium2 NeuronCores (v2).

Sharding: core c = (sequence s = c // 4, head-group g = c % 4); each core
computes its sequence's activations for its 4 heads and a partial o_proj;
the host sums the 4 head-group partials per sequence.

v2 structural changes over the f32r baseline:
  - q_a @ q_b fused on the host into one projection W_qf = W_qa (ln*W_qb)
    (the per-token rmsnorm scale commutes through the up-projection), so
    the 1536-wide q_a intermediate never exists on-chip.  The rms stats
    still need ||hs @ W_qa|| per token; that work is split 4 ways across
    the head-group cores (each takes one 512-token chunk, fed as its own
    input tensor) and the [1,512] 1/rms vectors are exchanged with an
    AllGather over the sequence group.
  - mixed precision tuned against the 2e-2 budget (measured 1.3e-2):
      fp8(e4m3) DoubleRow matmuls (2 contraction tiles/pass, 2x rate):
        rms-stats, fused q (hi + same-scale residual lo), kv_a rope part,
        kv_b K part, attention scores (nope+rope packed in the two slots)
      bf16 (full rate, half the SBUF/DMA of f32r):
        kv_a rank part, kv_b V part, PV, o_proj
    Value-critical paths (V, PV, o_proj) stay bf16; softmax-normalized
    paths (q, k, scores) take fp8.
  - K^T/Q live in SBUF in the DoubleRow pair layout [128, 2, S] (slot 0 =
    nope, slot 1 = rope(64)+zeros), so one fp8 matmul per 128-key tile
    yields the full 192-dim scores.  Only V round-trips through DRAM.
All fp8 scales are static powers of two with >=2x headroom.
"""

import numpy as np


def _ensure_concourse():
    try:
        import concourse  # noqa: F401
    except ImportError:
        import sys

        for p in ("/opt/trn_rl_repo", "/root/.axon_site/_ro/trn_rl_repo"):
            if p not in sys.path:
                sys.path.insert(0, p)


_ensure_concourse()

import concourse.bass as bass  # noqa: E402,F401
import concourse.bacc as bacc  # noqa: E402
import concourse.mybir as mybir  # noqa: E402
import concourse.tile as tile  # noqa: E402

F32 = mybir.dt.float32
F32R = mybir.dt.float32r
BF16 = mybir.dt.bfloat16
F8 = mybir.dt.float8e4
AF = mybir.ActivationFunctionType
DR = mybir.MatmulPerfMode.DoubleRow
NP_F8 = mybir.dt.np(F8)
NP_BF = mybir.dt.np(BF16)

# Problem constants (hardcoded per spec)
H = 16
HPC = 4
NC_CORES = 8
NOPE = 128
ROPE = 64
VD = 128
RANK = 512
HEAD = NOPE + ROPE
D = 2048
QA = 1536
T_FULL = 4096
B = 2
S_FULL = T_FULL // B
SCALE = float(HEAD) ** -0.5
EPS = 1e-6
NEG = -1.0e30

P = 128
KD = D // P         # 16 hidden k-tiles (8 DoubleRow pairs)
NPR = KD // 2       # 8 pairs
QF = HPC * HEAD     # 768 fused-q cols per core
MQ = QF // P        # 6 fused-q m-tiles
NT = S_FULL // 512  # 4 chunks
KR = RANK // P      # 4

# fp8 scales (pow2, ~2x headroom over measured maxima on the seed data)
S_HX = 16.0
S_WQA = 1024.0
S_WQF = 1024.0
S_WKP = 1024.0
S_CKV = 16.0
S_WBK = 1024.0
S_Q = 16.0
S_K = 16.0
EXP_SCALE = SCALE / (S_Q * S_K)
F8MAX = 240.0


def build_program(S=S_FULL):
    NQB = S // 512

    nc = bacc.Bacc("TRN2", target_bir_lowering=False, debug=False,
                   num_devices=NC_CORES)

    # ---- I/O (host pre-arranges weights into SBUF layouts) ----
    hsb = nc.dram_tensor("hsb", [P, KD, S], BF16, kind="ExternalInput").ap()
    hs8 = nc.dram_tensor("hs8", [P, KD, S], F8, kind="ExternalInput").ap()
    hst8 = nc.dram_tensor("hst8", [P, NPR, 2, 512], F8,
                          kind="ExternalInput").ap()
    wqa8 = nc.dram_tensor("wqa8", [P, NPR, 2, QA], F8,
                          kind="ExternalInput").ap()
    wqfh = nc.dram_tensor("wqfh", [P, NPR, 2, QF], F8,
                          kind="ExternalInput").ap()
    wqfl = nc.dram_tensor("wqfl", [P, NPR, 2, QF], F8,
                          kind="ExternalInput").ap()
    wkv = nc.dram_tensor("wkv", [P, KD, RANK], BF16, kind="ExternalInput").ap()
    wkp8 = nc.dram_tensor("wkp8", [P, NPR, 2, ROPE], F8,
                          kind="ExternalInput").ap()
    wbk8 = nc.dram_tensor("wbk8", [P, 2, 2, HPC * NOPE], F8,
                          kind="ExternalInput").ap()
    wbv = nc.dram_tensor("wbv", [P, KR, HPC * VD], BF16,
                         kind="ExternalInput").ap()
    wo = nc.dram_tensor("wo", [P, HPC, D], BF16, kind="ExternalInput").ap()
    csq = nc.dram_tensor("csq", [P, S], BF16, kind="ExternalInput").ap()
    snq = nc.dram_tensor("snq", [P, S], BF16, kind="ExternalInput").ap()
    masks = nc.dram_tensor("masks", [P, 4, 512], BF16, kind="ExternalInput").ap()
    out = nc.dram_tensor("out", [S, D], F32, kind="ExternalOutput").ap()

    # DRAM scratch
    ag_src = nc.dram_tensor("ag_src", [1, 512], F32R).ap()
    ag_dst = nc.dram_tensor("ag_dst", [1, HPC * 512], F32R).ap()

    with tile.TileContext(nc) as tc:
      with tc.tile_pool(name="persist", bufs=1) as persist:
        ones_f = persist.tile([P, 1], F32)
        ones_rf = persist.tile([1, P], F32)
        ones_col_r = persist.tile([P, 1], F32R)   # partition-sum lhsT
        ones_col_b = persist.tile([P, 1], BF16)   # lsum lhsT (bf16)
        ones_row_r = persist.tile([1, P], F32R)   # partition-broadcast lhsT
        zero_col = persist.tile([P, 1], F32)
        eps1 = persist.tile([1, 1], F32)
        nc.any.memset(ones_f[:], 1.0)
        nc.any.memset(ones_rf[:], 1.0)
        nc.any.memset(zero_col[:], 0.0)
        nc.any.memset(eps1[:], EPS)
        nc.scalar.activation(ones_col_r[:], ones_f[:], AF.Copy)
        nc.scalar.activation(ones_col_b[:], ones_f[:], AF.Copy)
        nc.scalar.activation(ones_row_r[:], ones_rf[:], AF.Copy)
        warm = persist.tile([1, 1], F32)
        nc.scalar.activation(warm[:], eps1[:], AF.Exp, bias=eps1[:])
        nc.scalar.activation(warm[:], eps1[:], AF.Sqrt, bias=eps1[:])
        nc.scalar.activation(warm[:], eps1[:], AF.Square)


        # ---- persistent fp8 pair-layout Q/K tiles (per 512-chunk) ----
        with tc.tile_pool(name="qk", bufs=1) as qkp:
          q2 = [[qkp.tile([P, 2, 512], F8, name=f"q2_{h}_{c}")
                 for c in range(NT)] for h in range(HPC)]
          kt2 = [[qkp.tile([P, 2, 512], F8, name=f"kt2_{h}_{c}")
                  for c in range(NT)] for h in range(HPC)]
          for h in range(HPC):
              for c in range(NT):
                  nc.any.memset(q2[h][c][ROPE:P, 1, :], 0.0)
                  nc.any.memset(kt2[h][c][ROPE:P, 1, :], 0.0)

          # kv_a rank weights go right-side; they persist through stage A
          s_aw = tc.alloc_tile_pool(name="s_aw", bufs=1, side="right")
          wkv_sb = [s_aw.tile([P, KD // 4, RANK], BF16, name=f"wkv{g}")
                    for g in range(4)]

          # =============== Stage S: rms stats + AllGather ================
          with (
              tc.tile_pool(name="stw", bufs=1) as stw,
              tc.tile_pool(name="ste", bufs=2) as ste,
              tc.tile_pool(name="stp", bufs=3, space="PSUM") as stp,
              tc.tile_pool(name="stps", bufs=1, space="PSUM") as stps,
          ):
              st_x = stw.tile([P, NPR, 2, 512], F8)
              st_wa = stw.tile([P, NPR, 2, QA // 2], F8)
              st_wb = stw.tile([P, NPR, 2, QA // 2], F8)
              nc.sync.dma_start(out=st_x[:], in_=hst8[:, :, :, :])
              for pr in range(NPR):
                  nc.sync.dma_start(out=st_wa[:, pr, :, :],
                                    in_=wqa8[:, pr, :, 0:QA // 2])
              for pr in range(NPR):
                  nc.sync.dma_start(out=st_wb[:, pr, :, :],
                                    in_=wqa8[:, pr, :, QA // 2:QA])
              sq_ps = stps.tile([1, 512], F32, name="st_sq")
              for m in range(QA // P):
                  st_w = st_wa if m < 6 else st_wb
                  mm = m if m < 6 else m - 6
                  ps = stp.tile([P, 512], F32, name="st_ps", tag="stmm")
                  for pr in range(NPR):
                      nc.tensor.matmul(
                          ps[:], st_w[:, pr, :, mm * P:(mm + 1) * P],
                          st_x[:, pr, :, :],
                          start=(pr == 0), stop=(pr == NPR - 1),
                          perf_mode=DR)
                  sq = ste.tile([P, 512], F32R, name="st_sqt", bufs=3)
                  nc.scalar.activation(sq[:], ps[:], AF.Square)
                  nc.tensor.matmul(sq_ps[:], ones_col_r[:], sq[:],
                                   start=(m == 0), stop=(m == QA // P - 1))
              std = ste.tile([1, 512], F32, name="st_std")
              nc.scalar.activation(std[:], sq_ps[:], AF.Sqrt,
                                   scale=1.0 / (QA * (S_HX * S_WQA * S_Q) ** 2),
                                   bias=eps1[:])
              rcp = ste.tile([1, 512], F32R, name="st_rcp")
              with nc.allow_low_precision(reason="f32r == f32 storage"):
                  nc.vector.reciprocal(rcp[:], std[:])
              nc.sync.dma_start(out=ag_src[:, :], in_=rcp[:])
              nc.gpsimd.collective_compute(
                  "AllGather", mybir.AluOpType.bypass,
                  replica_groups=[[0, 1, 2, 3], [4, 5, 6, 7]],
                  ins=[ag_src[:, :]], outs=[ag_dst[:, :]],
              )

          for k in range(KD):
              nc.sync.dma_start(out=wkv_sb[k // 4][:, k % 4, :],
                                in_=wkv[:, k, :])
          # SBUF-resident V / o_proj weights / masks (span stages A..C)
          bspan = tc.alloc_tile_pool(name="bspan", bufs=1)
          v_sb = [bspan.tile([P, 4, HPC * VD], BF16, name=f"v_sb{c}")
                  for c in range(NT)]
          wo_sb = bspan.tile([P, HPC, D], BF16)
          mask_sb = bspan.tile([P, 4, 512], BF16)

          # ============ Stage A: fused q + kv per 512-chunk ==============
          with (
              tc.tile_pool(name="aw", bufs=1) as aw,
              tc.tile_pool(name="ax", bufs=2) as ax,
              tc.tile_pool(name="ax8", bufs=2) as ax8,
              tc.tile_pool(name="aqr", bufs=1) as aqr,
              tc.tile_pool(name="ae", bufs=1) as ae,
              tc.tile_pool(name="ac", bufs=1) as ac,
              tc.tile_pool(name="ap2", bufs=2, space="PSUM") as ap2,
              tc.tile_pool(name="apc", bufs=2, space="PSUM") as apc,
              tc.tile_pool(name="apk", bufs=2, space="PSUM") as apk,
              tc.tile_pool(name="ape", bufs=1, space="PSUM") as ape,
              tc.tile_pool(name="aps", bufs=1, space="PSUM") as aps,
          ):
              def load_chunk(t):
                  ts = slice(t * 512, t * 512 + 512)
                  hx4 = [ax.tile([P, KD // 4, 512], BF16, name=f"hx{i}",
                                 tag=f"hx{i}") for i in range(4)]
                  x82 = [ax8.tile([P, KD // 2, 512], F8, name=f"hx8{i}",
                                  tag=f"hx8{i}") for i in range(2)]
                  for i in range(4):
                      ks = slice(i * (KD // 4), (i + 1) * (KD // 4))
                      nc.sync.dma_start(out=hx4[i][:], in_=hsb[:, ks, ts])
                  for i in range(2):
                      ks = slice(i * (KD // 2), (i + 1) * (KD // 2))
                      nc.sync.dma_start(out=x82[i][:], in_=hs8[:, ks, ts])
                  cs = ax8.tile([P, 512], BF16, name="cs", tag="cs")
                  sn = ax8.tile([P, 512], BF16, name="sn", tag="sn")
                  nc.sync.dma_start(out=cs[:], in_=csq[:, ts])
                  nc.sync.dma_start(out=sn[:], in_=snq[:, ts])
                  return hx4, x82, cs, sn

              cur = load_chunk(0)
              wqf_sb = [aw.tile([P, 2, QF], F8, name=f"wqfh{pr}")
                        for pr in range(NPR)]
              wqfl_sb = [aw.tile([P, 2, QF], F8, name=f"wqfl{pr}")
                         for pr in range(NPR)]
              wkp_sb = [aw.tile([P, 2, ROPE], F8, name=f"wkp{pr}")
                        for pr in range(NPR)]
              wbk_sb = [aw.tile([P, 2, HPC * NOPE], F8, name=f"wbk{pr}")
                        for pr in range(2)]
              wbv_sb = aw.tile([P, KR, HPC * VD], BF16)
              for pr in range(NPR):
                  nc.sync.dma_start(out=wkp_sb[pr][:], in_=wkp8[:, pr, :, :])
              for pr in range(2):
                  nc.sync.dma_start(out=wbk_sb[pr][:], in_=wbk8[:, pr, :, :])
              nc.sync.dma_start(out=wbv_sb[:], in_=wbv[:, :, :])
              for pr in range(NPR):
                  nc.sync.dma_start(out=wqf_sb[pr][:], in_=wqfh[:, pr, :, :])
              for pr in range(NPR):
                  nc.sync.dma_start(out=wqfl_sb[pr][:], in_=wqfl[:, pr, :, :])
              for t in range(NT):
                  ts = slice(t * 512, t * 512 + 512)
                  hx4, x82, cs_c, sn_c = cur
                  if t + 1 < NT:
                      cur = load_chunk(t + 1)

                  # ---- kv_a rank (bf16): evict raw, normalize in place --
                  ckv8 = ac.tile([P, KR, 512], F8, name="ckv8")
                  ckvb = ac.tile([P, KR, 512], BF16, name="ckvb")
                  sq_ps = aps.tile([1, 512], F32, name="kv_sq")
                  for m in range(KR):
                      ps = apc.tile([P, 512], F32, name="ckv_ps", tag="ckv")
                      for k in range(KD):
                          nc.tensor.matmul(
                              ps[:], wkv_sb[k // 4][:, k % 4,
                                            m * P:(m + 1) * P],
                              hx4[k // 4][:, k % 4, :],
                              start=(k == 0), stop=(k == KD - 1))
                      sq = ae.tile([P, 512], F32R, name="kv_sqt", bufs=1)
                      nc.scalar.activation(sq[:], ps[:], AF.Square)
                      nc.tensor.matmul(sq_ps[:], ones_col_r[:], sq[:],
                                       start=(m == 0), stop=(m == KR - 1))
                      nc.scalar.activation(ckv8[:, m, :], ps[:], AF.Copy,
                                           scale=S_CKV)
                      nc.scalar.activation(ckvb[:, m, :], ps[:], AF.Copy)
                  std = ae.tile([1, 512], F32, name="kv_std")
                  nc.scalar.activation(std[:], sq_ps[:], AF.Sqrt,
                                       scale=1.0 / RANK, bias=eps1[:])
                  rkv_r = ae.tile([1, 512], F32R, name="kv_rcp_r")
                  with nc.allow_low_precision(reason="f32r == f32 storage"):
                      nc.vector.reciprocal(rkv_r[:], std[:])
                  rbc = ae.tile([P, 512], F32R, name="kv_rbc")
                  nc.gpsimd.partition_broadcast(rbc[:], rkv_r[:])
                  for m in range(KR):
                      nc.vector.tensor_mul(ckv8[:, m, :], ckv8[:, m, :],
                                           rbc[:])
                      nc.vector.tensor_mul(ckvb[:, m, :], ckvb[:, m, :],
                                           rbc[:])

                  # ---- fused q (fp8 DR, hi + same-scale lo) ----
                  q_raw = aqr.tile([P, MQ, 512], BF16, name="q_raw", bufs=1)
                  for m in range(MQ):
                      ps = ap2.tile([P, 512], F32, name="q_ps", tag="qmm")
                      for pr in range(NPR):
                          nc.tensor.matmul(
                              ps[:], wqf_sb[pr][:, :, m * P:(m + 1) * P],
                              x82[pr // 4][:, (2 * pr) % NPR:
                                           (2 * pr) % NPR + 2, :],
                              start=(pr == 0), stop=False, perf_mode=DR)
                      for pr in range(NPR):
                          nc.tensor.matmul(
                              ps[:], wqfl_sb[pr][:, :, m * P:(m + 1) * P],
                              x82[pr // 4][:, (2 * pr) % NPR:
                                           (2 * pr) % NPR + 2, :],
                              start=False, stop=(pr == NPR - 1),
                              perf_mode=DR)
                      nc.scalar.activation(q_raw[:, m, :], ps[:], AF.Copy,
                                           scale=1.0 / (S_HX * S_WQF))

                  # ---- kv_a rope (fp8 DR) -> k_pe into kt2 slot 1 ----
                  ps_pe = ape.tile([ROPE, 512], F32, name="pe_ps")
                  for pr in range(NPR):
                      nc.tensor.matmul(
                          ps_pe[:], wkp_sb[pr][:, :, :],
                          x82[pr // 4][:, (2 * pr) % NPR:(2 * pr) % NPR + 2, :],
                          start=(pr == 0), stop=(pr == NPR - 1),
                          perf_mode=DR)
                  pe_raw = ae.tile([ROPE, 512], F32, name="pe_raw")
                  nc.scalar.activation(pe_raw[:], ps_pe[:], AF.Copy,
                                       scale=S_K / (S_HX * S_WKP))
                  pe_o = ae.tile([32, 512], F32, name="pe_o")
                  nc.sync.dma_start(out=pe_o[:], in_=pe_raw[32:ROPE, :])
                  ta = ae.tile([P, 512], F32, name="q_t1")[0:32, :]
                  tb = ae.tile([P, 512], F32, name="q_t2")[0:32, :]
                  tc_ = ae.tile([P, 512], F32, name="q_top")[0:32, :]
                  td = ae.tile([P, 512], F32, name="q_bot")[0:32, :]
                  nc.vector.tensor_mul(ta[:], pe_raw[0:32, :], cs_c[0:32, :])
                  nc.vector.tensor_mul(tb[:], pe_o[:], sn_c[0:32, :])
                  nc.vector.tensor_mul(tc_[:], pe_o[:], cs_c[0:32, :])
                  nc.vector.tensor_mul(td[:], pe_raw[0:32, :], sn_c[0:32, :])
                  for h in range(HPC):
                      nc.vector.tensor_sub(kt2[h][t][0:32, 1, :], ta[:], tb[:])
                      nc.vector.tensor_add(kt2[h][t][32:ROPE, 1, :],
                                           tc_[:], td[:])

                  # ---- kv_b K (fp8 DR) -> kt2 slot 0 ----
                  for h in range(HPC):
                      ps = apk.tile([P, 512], F32, name="k_ps", tag="kvb")
                      for pr in range(2):
                          nc.tensor.matmul(
                              ps[:], wbk_sb[pr][:, :, h * NOPE:(h + 1) * NOPE],
                              ckv8[:, 2 * pr:2 * pr + 2, :],
                              start=(pr == 0), stop=(pr == 1), perf_mode=DR)
                      nc.scalar.activation(kt2[h][t][:, 0, :], ps[:],
                                           AF.Copy,
                                           scale=S_K / (S_CKV * S_WBK))

                  # ---- kv_b V (bf16) token-major, straight into SBUF ----
                  for tt in range(4):
                      ps = apk.tile([P, HPC * VD], F32, name="v_ps", tag="kvb")
                      for k in range(KR):
                          nc.tensor.matmul(
                              ps[:], ckvb[:, k, tt * P:(tt + 1) * P],
                              wbv_sb[:, k, :], start=(k == 0),
                              stop=(k == KR - 1))
                      nc.scalar.activation(v_sb[t][:, tt, :], ps[:],
                                           AF.Copy)
                  if t == 2:
                      nc.sync.dma_start(out=mask_sb[:], in_=masks[:])
                      for h in range(HPC):
                          nc.sync.dma_start(out=wo_sb[:, h, :],
                                            in_=wo[:, h, :])

                  # ---- rs broadcast (per chunk) + q2 build ----
                  rsf = ae.tile([1, 512], F32R, name="rs_f")
                  nc.sync.dma_start(out=rsf[:], in_=ag_dst[:, ts])
                  rsq_bc = ae.tile([P, 512], F32R, name="rsq_bc")
                  nc.gpsimd.partition_broadcast(rsq_bc[:], rsf[:])
                  for h in range(HPC):
                      nc.vector.tensor_mul(q2[h][t][:, 0, :], q_raw[:, h, :],
                                           rsq_bc[:])
                  t1 = ae.tile([P, 512], F32, name="q_t1")
                  t2 = ae.tile([P, 512], F32, name="q_t2")
                  top = ae.tile([P, 512], F32, name="q_top")
                  bot = ae.tile([P, 512], F32, name="q_bot")
                  nc.vector.tensor_mul(t1[:], q_raw[:, 4, :], cs_c[:])
                  nc.vector.tensor_mul(t2[:], q_raw[:, 5, :], sn_c[:])
                  nc.vector.tensor_sub(top[:], t1[:], t2[:])
                  nc.vector.tensor_mul(t1[:], q_raw[:, 5, :], cs_c[:])
                  nc.vector.tensor_mul(t2[:], q_raw[:, 4, :], sn_c[:])
                  nc.vector.tensor_add(bot[:], t1[:], t2[:])
                  for h in range(HPC):
                      hrows = slice(32 * h, 32 * h + 32)
                      nc.vector.tensor_mul(q2[h][t][0:32, 1, :], top[hrows, :],
                                           rsq_bc[hrows, :])
                      nc.vector.tensor_mul(q2[h][t][32:ROPE, 1, :],
                                           bot[hrows, :], rsq_bc[hrows, :])

          s_aw.release()
          # ==== Stage B+C: attention sw-pipelined across heads + o_proj ====
          with (
              tc.tile_pool(name="bot", bufs=2) as botp,
              tc.tile_pool(name="be", bufs=3) as bep,
              tc.tile_pool(name="bt", bufs=3) as bt,
              tc.tile_pool(name="ce", bufs=4) as ce,
              tc.tile_pool(name="bp", bufs=2, space="PSUM") as bp,
              tc.tile_pool(name="bacc", bufs=2, space="PSUM") as bac,
              tc.tile_pool(name="bpl", bufs=1, space="PSUM") as bpl,
          ):
              def emit_pair(cur, kp):
                  qb, h, e_t, nk = cur["qb"], cur["h"], cur["e_t"], cur["nk"]
                  s2 = bp.tile([P, 2, 512], F32, name="s2", tag="s2")
                  for j in range(2):
                      kt = 2 * kp + j
                      kl = slice((kt % 4) * P, (kt % 4) * P + P)
                      nc.tensor.matmul(s2[:, j, :], kt2[h][kt // 4][:, :, kl],
                                       q2[h][qb][:, :, :],
                                       start=True, stop=True,
                                       perf_mode=DR)
                  dg = 2 * kp - (nk - 4)
                  if dg >= 0:
                      for j in range(2):
                          w = (dg + j + 1) * P
                          nc.vector.tensor_add(s2[:, j, 0:w], s2[:, j, 0:w],
                                               mask_sb[:, dg + j, 0:w])
                  nc.scalar.activation(e_t[:, 2 * kp:2 * kp + 2, :],
                                       s2[:, :, :], AF.Exp, bias=zero_col[:],
                                       scale=EXP_SCALE)

              def emit_pv(prev, kp):
                  h, e_t, nk = prev["h"], prev["e_t"], prev["nk"]
                  if kp == 0:
                      prev["l_ps"] = bpl.tile([1, 512], F32, name="l_ps")
                      prev["o_ps"] = bac.tile([P, 512], F32, name="o_ps",
                                              tag="acc")
                  for j in range(2):
                      kt = 2 * kp + j
                      nc.tensor.matmul(prev["l_ps"][:], ones_col_b[:],
                                       e_t[:, kt, :], start=(kt == 0),
                                       stop=(kt == nk - 1))
                      nc.tensor.matmul(prev["o_ps"][:],
                                       v_sb[kt // 4][:, kt % 4,
                                                     h * VD:(h + 1) * VD],
                                       e_t[:, kt, :], start=(kt == 0),
                                       stop=(kt == nk - 1))

              def emit_epilogue(prev):
                  linv_r = bt.tile([1, 512], F32R, name="linv_r")
                  with nc.allow_low_precision(reason="f32r == f32 storage"):
                      nc.vector.reciprocal(linv_r[:], prev["l_ps"][:])
                  lbc = bt.tile([P, 512], F32R, bufs=3, name="lbc")
                  nc.gpsimd.partition_broadcast(lbc[:], linv_r[:])
                  oth = botp.tile([P, 512], BF16, name=f"ot{prev['h']}")
                  nc.vector.tensor_mul(oth[:], prev["o_ps"][:], lbc[:])
                  return oth

              def emit_oproj(qb, ot4):
                  for tt in range(4):
                      tsl = slice(tt * P, tt * P + P)
                      for n in range(D // 512):
                          ps5 = bac.tile([P, 512], F32, name="ps5", tag="acc")
                          for h in range(HPC):
                              nc.tensor.matmul(
                                  ps5[:], ot4[h][:, tsl],
                                  wo_sb[:, h, n * 512:(n + 1) * 512],
                                  start=(h == 0), stop=(h == HPC - 1))
                          ev = ce.tile([P, 512], F32, name="ev5", bufs=4)
                          nc.vector.tensor_scalar_mul(ev[:], ps5[:], 1.0)
                          nc.sync.dma_start(
                              out=out[qb * 512 + tt * P:
                                      qb * 512 + (tt + 1) * P,
                                      n * 512:(n + 1) * 512],
                              in_=ev[:])

              prev = None
              ot4 = []
              done_qb = []
              for qb in range(NQB):
                  for h in range(HPC):
                      nk = 4 * (qb + 1)
                      cur = dict(qb=qb, h=h, nk=nk,
                                 e_t=bep.tile([P, S // P, 512], BF16,
                                              name="e_t", tag="e_t"))
                      np_prev = prev["nk"] // 2 if prev else 0
                      for kp in range(max(nk // 2, np_prev)):
                          if kp < nk // 2:
                              emit_pair(cur, kp)
                          if prev is not None and kp < np_prev:
                              emit_pv(prev, kp)
                      if prev is not None:
                          ot4.append(emit_epilogue(prev))
                          if len(ot4) == HPC:
                              done_qb.append((prev["qb"], ot4))
                              ot4 = []
                      if done_qb and h == 1:
                          q_, o_ = done_qb.pop(0)
                          emit_oproj(q_, o_)
                      prev = cur
              for kp in range(prev["nk"] // 2):
                  emit_pv(prev, kp)
              ot4.append(emit_epilogue(prev))
              done_qb.append((prev["qb"], ot4))
              for q_, o_ in done_qb:
                  emit_oproj(q_, o_)
          bspan.release()
    nc.compile()
    return nc


# ======================= host-side preparation =======================

def _pairs(a):
    """[D, M] -> [P, D//256, 2, M] DoubleRow pair layout."""
    Dd, M = a.shape
    return np.ascontiguousarray(
        a.reshape(Dd // 256, 2, P, M).transpose(2, 0, 1, 3))


def _q8(a, s):
    return np.clip(np.asarray(a, np.float32) * s,
                   -F8MAX, F8MAX).astype(NP_F8)


def shard_inputs(inputs, S=S_FULL):
    hs = np.asarray(inputs["hidden_states"], np.float32)
    cos = np.asarray(inputs["cos"], np.float32)
    sin = np.asarray(inputs["sin"], np.float32)
    w_q_a = np.asarray(inputs["w_q_a"], np.float32)
    q_ln = np.asarray(inputs["q_a_ln_w"], np.float32)
    w_q_b = np.asarray(inputs["w_q_b"], np.float32)
    w_kv_a = np.asarray(inputs["w_kv_a"], np.float32)
    kv_ln = np.asarray(inputs["kv_a_ln_w"], np.float32)
    w_kv_b = np.asarray(inputs["w_kv_b"], np.float32)
    w_o = np.asarray(inputs["w_o"], np.float32)

    nseq = hs.shape[0] // S

    # fold ln into the up-projections; fuse q_a @ q_b on the host
    wqb = q_ln[:, None] * w_q_b                    # [QA, H*HEAD]
    wkvb = kv_ln[:, None] * w_kv_b                 # [RANK, H*(NOPE+VD)]
    wqf_full = w_q_a @ wqb                         # [D, H*HEAD]
    wqf_h = wqf_full.reshape(D, H, HEAD)
    wkvb_h = wkvb.reshape(RANK, H, NOPE + VD)

    # shared (head-group independent) tensors
    wqa8 = _pairs(_q8(w_q_a, S_WQA))               # stats weights
    kva_pe = w_kv_a[:, RANK:]
    wkp_de = np.concatenate([kva_pe[:, 0::2], kva_pe[:, 1::2]], axis=1)
    wkp8 = _pairs(_q8(wkp_de, S_WKP))
    wkv_b16 = np.ascontiguousarray(
        w_kv_a[:, :RANK].reshape(KD, P, RANK).transpose(1, 0, 2)).astype(NP_BF)

    kl = np.arange(P)[:, None]
    ql = np.arange(512)[None, :]
    masks = np.stack(
        [np.where(P * r + kl <= ql, 0.0, NEG).astype(np.float32)
         for r in range(4)], axis=1).astype(NP_BF)  # [128, 4, 512]

    hs_bf = hs.astype(NP_BF)                       # bf16 master copy
    hs_f32 = hs_bf.astype(np.float32)

    in_maps = []
    for c in range(NC_CORES):
        s, g = c // 4, c % 4
        heads = slice(4 * g, 4 * g + 4)
        tok = slice(s * S, (s + 1) * S) if s < nseq else slice(0, S)
        hsT = hs_f32[tok].T                        # [D, S] (bf16-rounded)
        hsb = np.ascontiguousarray(
            hsT.reshape(KD, P, S).transpose(1, 0, 2)).astype(NP_BF)
        hs8 = np.ascontiguousarray(
            _q8(hsT, S_HX).reshape(KD, P, S).transpose(1, 0, 2))
        st = slice(g * 512, g * 512 + 512)
        hst8 = np.ascontiguousarray(
            _q8(hsT[:, st], S_HX).reshape(NPR, 2, P, 512).transpose(2, 0, 1, 3))

        # fused q: columns [h0n h1n h2n h3n | evens(4hx32) | odds(4hx32)]
        wn = wqf_h[:, heads, :NOPE].reshape(D, HPC * NOPE)
        pe = wqf_h[:, heads, NOPE:]                # [D, 4, 64]
        wev = pe[:, :, 0::2].reshape(D, HPC * 32)
        wod = pe[:, :, 1::2].reshape(D, HPC * 32)
        wqf_cols = np.concatenate([wn, wev, wod], axis=1)  # [D, 768]
        hi = _q8(wqf_cols, S_WQF)
        lo = _q8(wqf_cols - hi.astype(np.float32) / S_WQF, S_WQF)
        wqfh = _pairs(hi)
        wqfl = _pairs(lo)

        wbk = wkvb_h[:, heads, :NOPE].reshape(RANK, HPC * NOPE)
        wbk8 = np.ascontiguousarray(
            _q8(wbk, S_WBK).reshape(2, 2, P, HPC * NOPE).transpose(2, 0, 1, 3))
        wbv = np.ascontiguousarray(
            wkvb_h[:, heads, NOPE:].reshape(KR, P, HPC * VD)
            .transpose(1, 0, 2)).astype(NP_BF)
        wo_g = np.ascontiguousarray(
            w_o[512 * g:512 * (g + 1), :].reshape(HPC, P, D)
            .transpose(1, 0, 2)).astype(NP_BF)

        csq = np.ascontiguousarray(np.tile(cos[tok].T, (4, 1))).astype(NP_BF)
        snq = np.ascontiguousarray(np.tile(sin[tok].T, (4, 1))).astype(NP_BF)
        in_maps.append({
            "hsb": hsb, "hs8": hs8, "hst8": hst8, "wqa8": wqa8,
            "wqfh": wqfh, "wqfl": wqfl, "wkv": wkv_b16, "wkp8": wkp8,
            "wbk8": wbk8, "wbv": wbv, "wo": wo_g,
            "csq": csq, "snq": snq, "masks": masks,
        })
    return in_maps


_PROGRAM_CACHE = {}
LAST_RESULTS = None


def kernel(**inputs):
    global LAST_RESULTS
    import os

    from concourse.bass_utils import run_bass_kernel_spmd

    bsz = int(np.asarray(inputs.get("batch_size", B)))
    assert bsz == B, f"kernel hardcoded for batch_size={B}, got {bsz}"

    if "nc" not in _PROGRAM_CACHE:
        _PROGRAM_CACHE["nc"] = build_program(S_FULL)
    nc = _PROGRAM_CACHE["nc"]

    in_maps = shard_inputs(inputs, S_FULL)
    trace = bool(int(os.environ.get("BASSK_TRACE", "0")))
    res = run_bass_kernel_spmd(nc, in_maps, list(range(NC_CORES)), trace=trace)
    LAST_RESULTS = res
    parts = [np.asarray(r["out"], np.float32) for r in res.results]
    halves = [
        parts[0] + parts[1] + parts[2] + parts[3],
        parts[4] + parts[5] + parts[6] + parts[7],
    ]
    return np.concatenate(halves, axis=0).astype(np.float32)


# revision 31
# speedup vs baseline: 1.0019x; 1.0019x over previous
"""DeepseekV2 MLA prefill attention on 8 Trainium2 NeuronCores (v2).

Sharding: core c = (sequence s = c // 4, head-group g = c % 4); each core
computes its sequence's activations for its 4 heads and a partial o_proj;
the host sums the 4 head-group partials per sequence.

v2 structural changes over the f32r baseline:
  - q_a @ q_b fused on the host into one projection W_qf = W_qa (ln*W_qb)
    (the per-token rmsnorm scale commutes through the up-projection), so
    the 1536-wide q_a intermediate never exists on-chip.  The rms stats
    still need ||hs @ W_qa|| per token; that work is split 4 ways across
    the head-group cores (each takes one 512-token chunk, fed as its own
    input tensor) and the [1,512] 1/rms vectors are exchanged with an
    AllGather over the sequence group.
  - mixed precision tuned against the 2e-2 budget (measured 1.3e-2):
      fp8(e4m3) DoubleRow matmuls (2 contraction tiles/pass, 2x rate):
        rms-stats, fused q (hi + same-scale residual lo), kv_a rope part,
        kv_b K part, attention scores (nope+rope packed in the two slots)
      bf16 (full rate, half the SBUF/DMA of f32r):
        kv_a rank part, kv_b V part, PV, o_proj
    Value-critical paths (V, PV, o_proj) stay bf16; softmax-normalized
    paths (q, k, scores) take fp8.
  - K^T/Q live in SBUF in the DoubleRow pair layout [128, 2, S] (slot 0 =
    nope, slot 1 = rope(64)+zeros), so one fp8 matmul per 128-key tile
    yields the full 192-dim scores.  Only V round-trips through DRAM.
All fp8 scales are static powers of two with >=2x headroom.
"""

import numpy as np


def _ensure_concourse():
    try:
        import concourse  # noqa: F401
    except ImportError:
        import sys

        for p in ("/opt/trn_rl_repo", "/root/.axon_site/_ro/trn_rl_repo"):
            if p not in sys.path:
                sys.path.insert(0, p)


_ensure_concourse()

import concourse.bass as bass  # noqa: E402,F401
import concourse.bacc as bacc  # noqa: E402
import concourse.mybir as mybir  # noqa: E402
import concourse.tile as tile  # noqa: E402

F32 = mybir.dt.float32
F32R = mybir.dt.float32r
BF16 = mybir.dt.bfloat16
F8 = mybir.dt.float8e4
AF = mybir.ActivationFunctionType
DR = mybir.MatmulPerfMode.DoubleRow
NP_F8 = mybir.dt.np(F8)
NP_BF = mybir.dt.np(BF16)

# Problem constants (hardcoded per spec)
H = 16
HPC = 4
NC_CORES = 8
NOPE = 128
ROPE = 64
VD = 128
RANK = 512
HEAD = NOPE + ROPE
D = 2048
QA = 1536
T_FULL = 4096
B = 2
S_FULL = T_FULL // B
SCALE = float(HEAD) ** -0.5
EPS = 1e-6
NEG = -1.0e30

P = 128
KD = D // P         # 16 hidden k-tiles (8 DoubleRow pairs)
NPR = KD // 2       # 8 pairs
QF = HPC * HEAD     # 768 fused-q cols per core
MQ = QF // P        # 6 fused-q m-tiles
NT = S_FULL // 512  # 4 chunks
KR = RANK // P      # 4

# fp8 scales (pow2, ~2x headroom over measured maxima on the seed data)
S_HX = 16.0
S_WQA = 1024.0
S_WQF = 1024.0
S_WKP = 1024.0
S_CKV = 16.0
S_WBK = 1024.0
S_Q = 16.0
S_K = 16.0
EXP_SCALE = SCALE / (S_Q * S_K)
F8MAX = 240.0


def build_program(S=S_FULL):
    NQB = S // 512

    nc = bacc.Bacc("TRN2", target_bir_lowering=False, debug=False,
                   num_devices=NC_CORES)

    # ---- I/O (host pre-arranges weights into SBUF layouts) ----
    hsb = nc.dram_tensor("hsb", [P, KD, S], BF16, kind="ExternalInput").ap()
    hs8 = nc.dram_tensor("hs8", [P, KD, S], F8, kind="ExternalInput").ap()
    hst8 = nc.dram_tensor("hst8", [P, NPR, 2, 512], F8,
                          kind="ExternalInput").ap()
    wqa8 = nc.dram_tensor("wqa8", [P, NPR, 2, QA], F8,
                          kind="ExternalInput").ap()
    wqfh = nc.dram_tensor("wqfh", [P, NPR, 2, QF], F8,
                          kind="ExternalInput").ap()
    wqfl = nc.dram_tensor("wqfl", [P, NPR, 2, QF], F8,
                          kind="ExternalInput").ap()
    wkv = nc.dram_tensor("wkv", [P, KD, RANK], BF16, kind="ExternalInput").ap()
    wkp8 = nc.dram_tensor("wkp8", [P, NPR, 2, ROPE], F8,
                          kind="ExternalInput").ap()
    wbk8 = nc.dram_tensor("wbk8", [P, 2, 2, HPC * NOPE], F8,
                          kind="ExternalInput").ap()
    wbv = nc.dram_tensor("wbv", [P, KR, HPC * VD], BF16,
                         kind="ExternalInput").ap()
    wo = nc.dram_tensor("wo", [P, HPC, D], BF16, kind="ExternalInput").ap()
    csq = nc.dram_tensor("csq", [P, S], BF16, kind="ExternalInput").ap()
    snq = nc.dram_tensor("snq", [P, S], BF16, kind="ExternalInput").ap()
    masks = nc.dram_tensor("masks", [P, 4, 512], BF16, kind="ExternalInput").ap()
    out = nc.dram_tensor("out", [S, D], F32, kind="ExternalOutput").ap()

    # DRAM scratch
    ag_src = nc.dram_tensor("ag_src", [1, 512], F32R).ap()
    ag_dst = nc.dram_tensor("ag_dst", [1, HPC * 512], F32R).ap()

    with tile.TileContext(nc) as tc:
      with tc.tile_pool(name="persist", bufs=1) as persist:
        ones_f = persist.tile([P, 1], F32)
        ones_rf = persist.tile([1, P], F32)
        ones_col_r = persist.tile([P, 1], F32R)   # partition-sum lhsT
        ones_col_b = persist.tile([P, 1], BF16)   # lsum lhsT (bf16)
        ones_row_r = persist.tile([1, P], F32R)   # partition-broadcast lhsT
        zero_col = persist.tile([P, 1], F32)
        eps1 = persist.tile([1, 1], F32)
        nc.any.memset(ones_f[:], 1.0)
        nc.any.memset(ones_rf[:], 1.0)
        nc.any.memset(zero_col[:], 0.0)
        nc.any.memset(eps1[:], EPS)
        nc.scalar.activation(ones_col_r[:], ones_f[:], AF.Copy)
        nc.scalar.activation(ones_col_b[:], ones_f[:], AF.Copy)
        nc.scalar.activation(ones_row_r[:], ones_rf[:], AF.Copy)
        warm = persist.tile([1, 1], F32)
        nc.scalar.activation(warm[:], eps1[:], AF.Exp, bias=eps1[:])
        nc.scalar.activation(warm[:], eps1[:], AF.Sqrt, bias=eps1[:])
        nc.scalar.activation(warm[:], eps1[:], AF.Square)


        # ---- persistent fp8 pair-layout Q/K tiles (per 512-chunk) ----
        with tc.tile_pool(name="qk", bufs=1) as qkp:
          q2 = [[qkp.tile([P, 2, 512], F8, name=f"q2_{h}_{c}")
                 for c in range(NT)] for h in range(HPC)]
          kt2 = [[qkp.tile([P, 2, 512], F8, name=f"kt2_{h}_{c}")
                  for c in range(NT)] for h in range(HPC)]
          for h in range(HPC):
              for c in range(NT):
                  nc.any.memset(q2[h][c][ROPE:P, 1, :], 0.0)
                  nc.any.memset(kt2[h][c][ROPE:P, 1, :], 0.0)

          # kv_a rank weights go right-side; they persist through stage A
          s_aw = tc.alloc_tile_pool(name="s_aw", bufs=1, side="right")
          wkv_sb = [s_aw.tile([P, KD // 4, RANK], BF16, name=f"wkv{g}")
                    for g in range(4)]

          # =============== Stage S: rms stats + AllGather ================
          with (
              tc.tile_pool(name="stw", bufs=1) as stw,
              tc.tile_pool(name="ste", bufs=2) as ste,
              tc.tile_pool(name="stp", bufs=3, space="PSUM") as stp,
              tc.tile_pool(name="stps", bufs=1, space="PSUM") as stps,
          ):
              st_x = stw.tile([P, NPR, 2, 512], F8)
              st_wa = stw.tile([P, NPR, 2, QA // 2], F8)
              st_wb = stw.tile([P, NPR, 2, QA // 2], F8)
              nc.sync.dma_start(out=st_x[:], in_=hst8[:, :, :, :])
              for pr in range(NPR):
                  nc.sync.dma_start(out=st_wa[:, pr, :, :],
                                    in_=wqa8[:, pr, :, 0:QA // 2])
              for pr in range(NPR):
                  nc.sync.dma_start(out=st_wb[:, pr, :, :],
                                    in_=wqa8[:, pr, :, QA // 2:QA])
              sq_ps = stps.tile([1, 512], F32, name="st_sq")
              for m in range(QA // P):
                  st_w = st_wa if m < 6 else st_wb
                  mm = m if m < 6 else m - 6
                  ps = stp.tile([P, 512], F32, name="st_ps", tag="stmm")
                  for pr in range(NPR):
                      nc.tensor.matmul(
                          ps[:], st_w[:, pr, :, mm * P:(mm + 1) * P],
                          st_x[:, pr, :, :],
                          start=(pr == 0), stop=(pr == NPR - 1),
                          perf_mode=DR)
                  sq = ste.tile([P, 512], F32R, name="st_sqt", bufs=3)
                  nc.scalar.activation(sq[:], ps[:], AF.Square)
                  nc.tensor.matmul(sq_ps[:], ones_col_r[:], sq[:],
                                   start=(m == 0), stop=(m == QA // P - 1))
              std = ste.tile([1, 512], F32, name="st_std")
              nc.scalar.activation(std[:], sq_ps[:], AF.Sqrt,
                                   scale=1.0 / (QA * (S_HX * S_WQA * S_Q) ** 2),
                                   bias=eps1[:])
              rcp = ste.tile([1, 512], F32R, name="st_rcp")
              with nc.allow_low_precision(reason="f32r == f32 storage"):
                  nc.vector.reciprocal(rcp[:], std[:])
              nc.sync.dma_start(out=ag_src[:, :], in_=rcp[:])
              nc.gpsimd.collective_compute(
                  "AllGather", mybir.AluOpType.bypass,
                  replica_groups=[[0, 1, 2, 3], [4, 5, 6, 7]],
                  ins=[ag_src[:, :]], outs=[ag_dst[:, :]],
              )

          for k in range(KD):
              nc.sync.dma_start(out=wkv_sb[k // 4][:, k % 4, :],
                                in_=wkv[:, k, :])
          # SBUF-resident V / o_proj weights / masks (span stages A..C)
          bspan = tc.alloc_tile_pool(name="bspan", bufs=1)
          v_sb = [bspan.tile([P, 4, HPC * VD], BF16, name=f"v_sb{c}")
                  for c in range(NT)]
          wo_sb = bspan.tile([P, HPC, D], BF16)
          mask_sb = bspan.tile([P, 4, 512], BF16)

          # ============ Stage A: fused q + kv per 512-chunk ==============
          with (
              tc.tile_pool(name="aw", bufs=1) as aw,
              tc.tile_pool(name="ax", bufs=2) as ax,
              tc.tile_pool(name="ax8", bufs=2) as ax8,
              tc.tile_pool(name="aqr", bufs=1) as aqr,
              tc.tile_pool(name="ae", bufs=1) as ae,
              tc.tile_pool(name="ac", bufs=1) as ac,
              tc.tile_pool(name="ap2", bufs=2, space="PSUM") as ap2,
              tc.tile_pool(name="apc", bufs=2, space="PSUM") as apc,
              tc.tile_pool(name="apk", bufs=2, space="PSUM") as apk,
              tc.tile_pool(name="ape", bufs=1, space="PSUM") as ape,
              tc.tile_pool(name="aps", bufs=1, space="PSUM") as aps,
          ):
              def load_chunk(t):
                  ts = slice(t * 512, t * 512 + 512)
                  hx4 = [ax.tile([P, KD // 4, 512], BF16, name=f"hx{i}",
                                 tag=f"hx{i}") for i in range(4)]
                  x82 = [ax8.tile([P, KD // 2, 512], F8, name=f"hx8{i}",
                                  tag=f"hx8{i}") for i in range(2)]
                  for i in range(4):
                      ks = slice(i * (KD // 4), (i + 1) * (KD // 4))
                      nc.sync.dma_start(out=hx4[i][:], in_=hsb[:, ks, ts])
                  for i in range(2):
                      ks = slice(i * (KD // 2), (i + 1) * (KD // 2))
                      nc.sync.dma_start(out=x82[i][:], in_=hs8[:, ks, ts])
                  cs = ax8.tile([P, 512], BF16, name="cs", tag="cs")
                  sn = ax8.tile([P, 512], BF16, name="sn", tag="sn")
                  nc.sync.dma_start(out=cs[:], in_=csq[:, ts])
                  nc.sync.dma_start(out=sn[:], in_=snq[:, ts])
                  return hx4, x82, cs, sn

              cur = load_chunk(0)
              wqf_sb = [aw.tile([P, 2, QF], F8, name=f"wqfh{pr}")
                        for pr in range(NPR)]
              wqfl_sb = [aw.tile([P, 2, QF], F8, name=f"wqfl{pr}")
                         for pr in range(NPR)]
              wkp_sb = [aw.tile([P, 2, ROPE], F8, name=f"wkp{pr}")
                        for pr in range(NPR)]
              wbk_sb = [aw.tile([P, 2, HPC * NOPE], F8, name=f"wbk{pr}")
                        for pr in range(2)]
              wbv_sb = aw.tile([P, KR, HPC * VD], BF16)
              for pr in range(NPR):
                  nc.sync.dma_start(out=wkp_sb[pr][:], in_=wkp8[:, pr, :, :])
              for pr in range(2):
                  nc.sync.dma_start(out=wbk_sb[pr][:], in_=wbk8[:, pr, :, :])
              nc.sync.dma_start(out=wbv_sb[:], in_=wbv[:, :, :])
              for pr in range(NPR):
                  nc.sync.dma_start(out=wqf_sb[pr][:], in_=wqfh[:, pr, :, :])
              for pr in range(NPR):
                  nc.sync.dma_start(out=wqfl_sb[pr][:], in_=wqfl[:, pr, :, :])
              for t in range(NT):
                  ts = slice(t * 512, t * 512 + 512)
                  hx4, x82, cs_c, sn_c = cur
                  if t + 1 < NT:
                      cur = load_chunk(t + 1)

                  # ---- kv_a rank (bf16): evict raw, normalize in place --
                  ckv8 = ac.tile([P, KR, 512], F8, name="ckv8")
                  ckvb = ac.tile([P, KR, 512], BF16, name="ckvb")
                  sq_ps = aps.tile([1, 512], F32, name="kv_sq")
                  for m in range(KR):
                      ps = apc.tile([P, 512], F32, name="ckv_ps", tag="ckv")
                      for k in range(KD):
                          nc.tensor.matmul(
                              ps[:], wkv_sb[k // 4][:, k % 4,
                                            m * P:(m + 1) * P],
                              hx4[k // 4][:, k % 4, :],
                              start=(k == 0), stop=(k == KD - 1))
                      sq = ae.tile([P, 512], F32R, name="kv_sqt", bufs=1)
                      nc.scalar.activation(sq[:], ps[:], AF.Square)
                      nc.tensor.matmul(sq_ps[:], ones_col_r[:], sq[:],
                                       start=(m == 0), stop=(m == KR - 1))
                      nc.scalar.activation(ckv8[:, m, :], ps[:], AF.Copy,
                                           scale=S_CKV)
                      nc.scalar.activation(ckvb[:, m, :], ps[:], AF.Copy)
                  std = ae.tile([1, 512], F32, name="kv_std")
                  nc.scalar.activation(std[:], sq_ps[:], AF.Sqrt,
                                       scale=1.0 / RANK, bias=eps1[:])
                  rkv_r = ae.tile([1, 512], F32R, name="kv_rcp_r")
                  with nc.allow_low_precision(reason="f32r == f32 storage"):
                      nc.vector.reciprocal(rkv_r[:], std[:])
                  rbc = ae.tile([P, 512], F32R, name="kv_rbc")
                  nc.gpsimd.partition_broadcast(rbc[:], rkv_r[:])
                  for m in range(KR):
                      nc.vector.tensor_mul(ckv8[:, m, :], ckv8[:, m, :],
                                           rbc[:])
                      nc.vector.tensor_mul(ckvb[:, m, :], ckvb[:, m, :],
                                           rbc[:])

                  # ---- fused q (fp8 DR, hi + same-scale lo) ----
                  q_raw = aqr.tile([P, MQ, 512], BF16, name="q_raw", bufs=1)
                  for m in range(MQ):
                      ps = ap2.tile([P, 512], F32, name="q_ps", tag="qmm")
                      for pr in range(NPR):
                          nc.tensor.matmul(
                              ps[:], wqf_sb[pr][:, :, m * P:(m + 1) * P],
                              x82[pr // 4][:, (2 * pr) % NPR:
                                           (2 * pr) % NPR + 2, :],
                              start=(pr == 0), stop=False, perf_mode=DR)
                      for pr in range(NPR):
                          nc.tensor.matmul(
                              ps[:], wqfl_sb[pr][:, :, m * P:(m + 1) * P],
                              x82[pr // 4][:, (2 * pr) % NPR:
                                           (2 * pr) % NPR + 2, :],
                              start=False, stop=(pr == NPR - 1),
                              perf_mode=DR)
                      nc.scalar.activation(q_raw[:, m, :], ps[:], AF.Copy,
                                           scale=1.0 / (S_HX * S_WQF))

                  # ---- kv_a rope (fp8 DR) -> k_pe into kt2 slot 1 ----
                  ps_pe = ape.tile([ROPE, 512], F32, name="pe_ps")
                  for pr in range(NPR):
                      nc.tensor.matmul(
                          ps_pe[:], wkp_sb[pr][:, :, :],
                          x82[pr // 4][:, (2 * pr) % NPR:(2 * pr) % NPR + 2, :],
                          start=(pr == 0), stop=(pr == NPR - 1),
                          perf_mode=DR)
                  pe_raw = ae.tile([ROPE, 512], F32, name="pe_raw")
                  nc.scalar.activation(pe_raw[:], ps_pe[:], AF.Copy,
                                       scale=S_K / (S_HX * S_WKP))
                  pe_o = ae.tile([32, 512], F32, name="pe_o")
                  nc.sync.dma_start(out=pe_o[:], in_=pe_raw[32:ROPE, :])
                  ta = ae.tile([P, 512], F32, name="q_t1")[0:32, :]
                  tb = ae.tile([P, 512], F32, name="q_t2")[0:32, :]
                  tc_ = ae.tile([P, 512], F32, name="q_top")[0:32, :]
                  td = ae.tile([P, 512], F32, name="q_bot")[0:32, :]
                  nc.vector.tensor_mul(ta[:], pe_raw[0:32, :], cs_c[0:32, :])
                  nc.vector.tensor_mul(tb[:], pe_o[:], sn_c[0:32, :])
                  nc.vector.tensor_mul(tc_[:], pe_o[:], cs_c[0:32, :])
                  nc.vector.tensor_mul(td[:], pe_raw[0:32, :], sn_c[0:32, :])
                  for h in range(HPC):
                      nc.vector.tensor_sub(kt2[h][t][0:32, 1, :], ta[:], tb[:])
                      nc.vector.tensor_add(kt2[h][t][32:ROPE, 1, :],
                                           tc_[:], td[:])

                  # ---- kv_b K (fp8 DR) -> kt2 slot 0 ----
                  for h in range(HPC):
                      ps = apk.tile([P, 512], F32, name="k_ps", tag="kvb")
                      for pr in range(2):
                          nc.tensor.matmul(
                              ps[:], wbk_sb[pr][:, :, h * NOPE:(h + 1) * NOPE],
                              ckv8[:, 2 * pr:2 * pr + 2, :],
                              start=(pr == 0), stop=(pr == 1), perf_mode=DR)
                      nc.scalar.activation(kt2[h][t][:, 0, :], ps[:],
                                           AF.Copy,
                                           scale=S_K / (S_CKV * S_WBK))

                  # ---- kv_b V (bf16) token-major, straight into SBUF ----
                  for tt in range(4):
                      ps = apk.tile([P, HPC * VD], F32, name="v_ps", tag="kvb")
                      for k in range(KR):
                          nc.tensor.matmul(
                              ps[:], ckvb[:, k, tt * P:(tt + 1) * P],
                              wbv_sb[:, k, :], start=(k == 0),
                              stop=(k == KR - 1))
                      nc.scalar.activation(v_sb[t][:, tt, :], ps[:],
                                           AF.Copy)
                  if t == 2:
                      nc.sync.dma_start(out=mask_sb[:], in_=masks[:])
                      for h in range(HPC):
                          nc.sync.dma_start(out=wo_sb[:, h, :],
                                            in_=wo[:, h, :])

                  # ---- rs broadcast (per chunk) + q2 build ----
                  rsf = ae.tile([1, 512], F32R, name="rs_f")
                  nc.sync.dma_start(out=rsf[:], in_=ag_dst[:, ts])
                  rsq_bc = ae.tile([P, 512], F32R, name="rsq_bc")
                  nc.gpsimd.partition_broadcast(rsq_bc[:], rsf[:])
                  for h in range(HPC):
                      nc.vector.tensor_mul(q2[h][t][:, 0, :], q_raw[:, h, :],
                                           rsq_bc[:])
                  t1 = ae.tile([P, 512], F32, name="q_t1")
                  t2 = ae.tile([P, 512], F32, name="q_t2")
                  top = ae.tile([P, 512], F32, name="q_top")
                  bot = ae.tile([P, 512], F32, name="q_bot")
                  nc.vector.tensor_mul(t1[:], q_raw[:, 4, :], cs_c[:])
                  nc.vector.tensor_mul(t2[:], q_raw[:, 5, :], sn_c[:])
                  nc.vector.tensor_sub(top[:], t1[:], t2[:])
                  nc.vector.tensor_mul(t1[:], q_raw[:, 5, :], cs_c[:])
                  nc.vector.tensor_mul(t2[:], q_raw[:, 4, :], sn_c[:])
                  nc.vector.tensor_add(bot[:], t1[:], t2[:])
                  for h in range(HPC):
                      hrows = slice(32 * h, 32 * h + 32)
                      nc.vector.tensor_mul(q2[h][t][0:32, 1, :], top[hrows, :],
                                           rsq_bc[hrows, :])
                      nc.vector.tensor_mul(q2[h][t][32:ROPE, 1, :],
                                           bot[hrows, :], rsq_bc[hrows, :])

          s_aw.release()
          # ==== Stage B+C: attention sw-pipelined across heads + o_proj ====
          with (
              tc.tile_pool(name="bot", bufs=2) as botp,
              tc.tile_pool(name="be", bufs=3) as bep,
              tc.tile_pool(name="bt", bufs=3) as bt,
              tc.tile_pool(name="ce", bufs=4) as ce,
              tc.tile_pool(name="bp", bufs=2, space="PSUM") as bp,
              tc.tile_pool(name="bacc", bufs=2, space="PSUM") as bac,
              tc.tile_pool(name="bpl", bufs=1, space="PSUM") as bpl,
          ):
              def emit_pair(cur, kp):
                  qb, h, e_t, nk = cur["qb"], cur["h"], cur["e_t"], cur["nk"]
                  s2 = bp.tile([P, 2, 512], F32, name="s2", tag="s2")
                  for j in range(2):
                      kt = 2 * kp + j
                      kl = slice((kt % 4) * P, (kt % 4) * P + P)
                      nc.tensor.matmul(s2[:, j, :], kt2[h][kt // 4][:, :, kl],
                                       q2[h][qb][:, :, :],
                                       start=True, stop=True,
                                       perf_mode=DR)
                  dg = 2 * kp - (nk - 4)
                  if dg >= 0:
                      for j in range(2):
                          w = (dg + j + 1) * P
                          nc.vector.tensor_add(s2[:, j, 0:w], s2[:, j, 0:w],
                                               mask_sb[:, dg + j, 0:w])
                  nc.scalar.activation(e_t[:, 2 * kp:2 * kp + 2, :],
                                       s2[:, :, :], AF.Exp, bias=zero_col[:],
                                       scale=EXP_SCALE)

              def emit_pv(prev, kp):
                  h, e_t, nk = prev["h"], prev["e_t"], prev["nk"]
                  if kp == 0:
                      prev["l_ps"] = bpl.tile([1, 512], F32, name="l_ps")
                      prev["o_ps"] = bac.tile([P, 512], F32, name="o_ps",
                                              tag="acc")
                  for j in range(2):
                      kt = 2 * kp + j
                      nc.tensor.matmul(prev["l_ps"][:], ones_col_b[:],
                                       e_t[:, kt, :], start=(kt == 0),
                                       stop=(kt == nk - 1))
                      nc.tensor.matmul(prev["o_ps"][:],
                                       v_sb[kt // 4][:, kt % 4,
                                                     h * VD:(h + 1) * VD],
                                       e_t[:, kt, :], start=(kt == 0),
                                       stop=(kt == nk - 1))

              def emit_epilogue(prev):
                  linv_r = bt.tile([1, 512], F32R, name="linv_r")
                  with nc.allow_low_precision(reason="f32r == f32 storage"):
                      nc.vector.reciprocal(linv_r[:], prev["l_ps"][:])
                  lbc = bt.tile([P, 512], F32R, bufs=3, name="lbc")
                  nc.gpsimd.partition_broadcast(lbc[:], linv_r[:])
                  oth = botp.tile([P, 512], BF16, name=f"ot{prev['h']}")
                  nc.vector.tensor_mul(oth[:], prev["o_ps"][:], lbc[:])
                  return oth

              def emit_oproj(qb, ot4, half=None):
                  tts = range(4) if half is None else (
                      range(2) if half == 0 else range(2, 4))
                  for tt in tts:
                      tsl = slice(tt * P, tt * P + P)
                      for n in range(D // 512):
                          ps5 = bac.tile([P, 512], F32, name="ps5", tag="acc")
                          for h in range(HPC):
                              nc.tensor.matmul(
                                  ps5[:], ot4[h][:, tsl],
                                  wo_sb[:, h, n * 512:(n + 1) * 512],
                                  start=(h == 0), stop=(h == HPC - 1))
                          ev = ce.tile([P, 512], F32, name="ev5", bufs=4)
                          nc.vector.tensor_scalar_mul(ev[:], ps5[:], 1.0)
                          nc.sync.dma_start(
                              out=out[qb * 512 + tt * P:
                                      qb * 512 + (tt + 1) * P,
                                      n * 512:(n + 1) * 512],
                              in_=ev[:])

              prev = None
              ot4 = []
              done_qb = []
              for qb in range(NQB):
                  for h in range(HPC):
                      nk = 4 * (qb + 1)
                      cur = dict(qb=qb, h=h, nk=nk,
                                 e_t=bep.tile([P, S // P, 512], BF16,
                                              name="e_t", tag="e_t"))
                      np_prev = prev["nk"] // 2 if prev else 0
                      for kp in range(max(nk // 2, np_prev)):
                          if kp < nk // 2:
                              emit_pair(cur, kp)
                          if prev is not None and kp < np_prev:
                              emit_pv(prev, kp)
                      if prev is not None:
                          ot4.append(emit_epilogue(prev))
                          if len(ot4) == HPC:
                              done_qb.append((prev["qb"], ot4))
                              ot4 = []
                      if done_qb and h == 1:
                          emit_oproj(done_qb[0][0], done_qb[0][1], half=0)
                      if done_qb and h == 2:
                          q_, o_ = done_qb.pop(0)
                          emit_oproj(q_, o_, half=1)
                      prev = cur
              for kp in range(prev["nk"] // 2):
                  emit_pv(prev, kp)
              ot4.append(emit_epilogue(prev))
              done_qb.append((prev["qb"], ot4))
              for q_, o_ in done_qb:
                  emit_oproj(q_, o_)
          bspan.release()
    nc.compile()
    return nc


# ======================= host-side preparation =======================

def _pairs(a):
    """[D, M] -> [P, D//256, 2, M] DoubleRow pair layout."""
    Dd, M = a.shape
    return np.ascontiguousarray(
        a.reshape(Dd // 256, 2, P, M).transpose(2, 0, 1, 3))


def _q8(a, s):
    return np.clip(np.asarray(a, np.float32) * s,
                   -F8MAX, F8MAX).astype(NP_F8)


def shard_inputs(inputs, S=S_FULL):
    hs = np.asarray(inputs["hidden_states"], np.float32)
    cos = np.asarray(inputs["cos"], np.float32)
    sin = np.asarray(inputs["sin"], np.float32)
    w_q_a = np.asarray(inputs["w_q_a"], np.float32)
    q_ln = np.asarray(inputs["q_a_ln_w"], np.float32)
    w_q_b = np.asarray(inputs["w_q_b"], np.float32)
    w_kv_a = np.asarray(inputs["w_kv_a"], np.float32)
    kv_ln = np.asarray(inputs["kv_a_ln_w"], np.float32)
    w_kv_b = np.asarray(inputs["w_kv_b"], np.float32)
    w_o = np.asarray(inputs["w_o"], np.float32)

    nseq = hs.shape[0] // S

    # fold ln into the up-projections; fuse q_a @ q_b on the host
    wqb = q_ln[:, None] * w_q_b                    # [QA, H*HEAD]
    wkvb = kv_ln[:, None] * w_kv_b                 # [RANK, H*(NOPE+VD)]
    wqf_full = w_q_a @ wqb                         # [D, H*HEAD]
    wqf_h = wqf_full.reshape(D, H, HEAD)
    wkvb_h = wkvb.reshape(RANK, H, NOPE + VD)

    # shared (head-group independent) tensors
    wqa8 = _pairs(_q8(w_q_a, S_WQA))               # stats weights
    kva_pe = w_kv_a[:, RANK:]
    wkp_de = np.concatenate([kva_pe[:, 0::2], kva_pe[:, 1::2]], axis=1)
    wkp8 = _pairs(_q8(wkp_de, S_WKP))
    wkv_b16 = np.ascontiguousarray(
        w_kv_a[:, :RANK].reshape(KD, P, RANK).transpose(1, 0, 2)).astype(NP_BF)

    kl = np.arange(P)[:, None]
    ql = np.arange(512)[None, :]
    masks = np.stack(
        [np.where(P * r + kl <= ql, 0.0, NEG).astype(np.float32)
         for r in range(4)], axis=1).astype(NP_BF)  # [128, 4, 512]

    hs_bf = hs.astype(NP_BF)                       # bf16 master copy
    hs_f32 = hs_bf.astype(np.float32)

    in_maps = []
    for c in range(NC_CORES):
        s, g = c // 4, c % 4
        heads = slice(4 * g, 4 * g + 4)
        tok = slice(s * S, (s + 1) * S) if s < nseq else slice(0, S)
        hsT = hs_f32[tok].T                        # [D, S] (bf16-rounded)
        hsb = np.ascontiguousarray(
            hsT.reshape(KD, P, S).transpose(1, 0, 2)).astype(NP_BF)
        hs8 = np.ascontiguousarray(
            _q8(hsT, S_HX).reshape(KD, P, S).transpose(1, 0, 2))
        st = slice(g * 512, g * 512 + 512)
        hst8 = np.ascontiguousarray(
            _q8(hsT[:, st], S_HX).reshape(NPR, 2, P, 512).transpose(2, 0, 1, 3))

        # fused q: columns [h0n h1n h2n h3n | evens(4hx32) | odds(4hx32)]
        wn = wqf_h[:, heads, :NOPE].reshape(D, HPC * NOPE)
        pe = wqf_h[:, heads, NOPE:]                # [D, 4, 64]
        wev = pe[:, :, 0::2].reshape(D, HPC * 32)
        wod = pe[:, :, 1::2].reshape(D, HPC * 32)
        wqf_cols = np.concatenate([wn, wev, wod], axis=1)  # [D, 768]
        hi = _q8(wqf_cols, S_WQF)
        lo = _q8(wqf_cols - hi.astype(np.float32) / S_WQF, S_WQF)
        wqfh = _pairs(hi)
        wqfl = _pairs(lo)

        wbk = wkvb_h[:, heads, :NOPE].reshape(RANK, HPC * NOPE)
        wbk8 = np.ascontiguousarray(
            _q8(wbk, S_WBK).reshape(2, 2, P, HPC * NOPE).transpose(2, 0, 1, 3))
        wbv = np.ascontiguousarray(
            wkvb_h[:, heads, NOPE:].reshape(KR, P, HPC * VD)
            .transpose(1, 0, 2)).astype(NP_BF)
        wo_g = np.ascontiguousarray(
            w_o[512 * g:512 * (g + 1), :].reshape(HPC, P, D)
            .transpose(1, 0, 2)).astype(NP_BF)

        csq = np.ascontiguousarray(np.tile(cos[tok].T, (4, 1))).astype(NP_BF)
        snq = np.ascontiguousarray(np.tile(sin[tok].T, (4, 1))).astype(NP_BF)
        in_maps.append({
            "hsb": hsb, "hs8": hs8, "hst8": hst8, "wqa8": wqa8,
            "wqfh": wqfh, "wqfl": wqfl, "wkv": wkv_b16, "wkp8": wkp8,
            "wbk8": wbk8, "wbv": wbv, "wo": wo_g,
            "csq": csq, "snq": snq, "masks": masks,
        })
    return in_maps


_PROGRAM_CACHE = {}
LAST_RESULTS = None


def kernel(**inputs):
    global LAST_RESULTS
    import os

    from concourse.bass_utils import run_bass_kernel_spmd

    bsz = int(np.asarray(inputs.get("batch_size", B)))
    assert bsz == B, f"kernel hardcoded for batch_size={B}, got {bsz}"

    if "nc" not in _PROGRAM_CACHE:
        _PROGRAM_CACHE["nc"] = build_program(S_FULL)
    nc = _PROGRAM_CACHE["nc"]

    in_maps = shard_inputs(inputs, S_FULL)
    trace = bool(int(os.environ.get("BASSK_TRACE", "0")))
    res = run_bass_kernel_spmd(nc, in_maps, list(range(NC_CORES)), trace=trace)
    LAST_RESULTS = res
    parts = [np.asarray(r["out"], np.float32) for r in res.results]
    halves = [
        parts[0] + parts[1] + parts[2] + parts[3],
        parts[4] + parts[5] + parts[6] + parts[7],
    ]
    return np.concatenate(halves, axis=0).astype(np.float32)
